# revision 23
# baseline (speedup 1.0000x reference)
"""AdaptiveFrequencyAsymmetricHuberLoss on 8 TRN2 NeuronCores (Bass/Tile).

loss = mean( wf(t) * asym(t, sign(e)) * huber(e, delta(t)) ),  e = p - t
  delta(t)   = 5 + 0.05 t
  w_under(t) = 1 + 0.05 t
  w_over(t)  = 2 exp(-t/10)
  wf(t)      = clip(3 / (freq[t] + 1), 1, 3)   (t integer 0..130)
  huber      = 0.5 cl (2e - cl), cl = clip(e, -delta, delta)   (exact identity)

Sharding: pure data parallel; each of the 8 cores streams a contiguous
1/8 of the elements as [128, 16384], DMA-cast f32->bf16 on load.

Per-tile pipeline:
  ACT:  nd = -delta,  ws = w_over (Exp)
  DVE:  e  = p - t                                   [bf16 2x]
        sh = |cl| * (2e - cl) = sign(e) * 2*huber    [8-op custom, 1x]
        shp = max(sh, 0), rm = max(-sh, 0)           [bf16 4x]
        wu = 1 + 0.05 t                              [bf16 4x]
        qo = shp * ws,  qu = rm * wu                 [bf16 2x]
  PE:   ones-colsum matmuls accumulate sum(qo)+sum(qu) into one
        [1,512] PSUM bank across all tiles (only the total matters).
Host divides by 2N and reduces in float64.

The freq table is handled host-side: wf >= 1 always, and wf > 1 only
for freq counts < 2, so the host enumerates the (usually zero) table
entries with wf > 1 and the kernel adds masked correction passes per
entry (accum_out into a separate SBUF accumulator).
"""

import contextlib

import numpy as np

import concourse.bass as bass
import concourse.dve_ops as dve_ops_mod
import concourse.tile as tile
from concourse import bacc, mybir
from concourse.bass_utils import run_bass_kernel_spmd
from concourse.dve_ops import DveOp
from concourse.dve_spec import (
    Spec,
    Src0,
    Src1,
    Zero,
    _has_src1,
    lower,
    maxx,
    minn,
)
from concourse.dve_uop import DveOpSpec

N = 16_777_216
NCORES = 8
P = 128
PER_CORE = N // NCORES          # 2_097_152
FREE = PER_CORE // P            # 16384
TILE_FS = [1024, 3072, 4096, 4096, 3072, 1024]
assert sum(TILE_FS) == FREE

LN2 = 0.6931471805599453

f32 = mybir.dt.float32
bf16 = mybir.dt.bfloat16


def _register_op(name, spec):
    for o in dve_ops_mod.OPS:
        if o.name == name:
            return o
    opcode = max(dve_ops_mod._SUB_OPCODE_FOR_NAME.values()) + 1
    assert opcode < 0x20, "custom-DVE opcode rows exhausted"
    shas = {}
    for ver in ("v3", "v4"):
        try:
            c = DveOpSpec(
                name=name, opcode=opcode, uops=lower(spec, ver=ver),
                rd1_en=_has_src1(spec),
            )
            shas[ver] = c.sha(ver)
        except Exception:
            pass
    op = DveOp(name, spec, subdim=False, uops_sha=shas)
    dve_ops_mod.OPS.append(op)
    dve_ops_mod.CUSTOM_DVE_SPECS[name] = spec
    dve_ops_mod._SUB_OPCODE_FOR_NAME[name] = opcode
    return op


def _huber_signed_ref(in0, in1, c0, c1, c2):
    e = in0.astype(np.float32)
    nd = in1.astype(np.float32)
    cl = np.minimum(np.maximum(e, nd), -nd)
    return (np.abs(cl) * ((e + e) - cl)).astype(np.float32)


# sh = |cl| * (2e - cl) = sign(e) * 2*huber(e, delta);  in0 = e, in1 = -delta
_dd = Zero - Src1
_cl = minn(maxx(Src0, Src1), _dd)
_v = (Src0 + Src0) - _cl
_acl = maxx(_cl, Zero - _cl)
HUBER_SIGNED_SPEC = Spec(
    body=_acl * _v,
    reference=_huber_signed_ref,
)

HUBER_SIGNED_OP = _register_op("HUBER_SIGNED_LOSS_ANT", HUBER_SIGNED_SPEC)


def build(corrections):
    """Build + compile the SPMD graph. corrections: tuple of (k, wf_k - 1)."""
    Alu = mybir.AluOpType
    Act = mybir.ActivationFunctionType

    nc = bacc.Bacc(
        "TRN2", target_bir_lowering=False, debug=False, num_devices=NCORES
    )

    # const AP for the Identity bias (-5); 0.0/1.0 pre-registered by Bass
    h = nc.alloc_sbuf_tensor("const-f32-neg5", [128, 1], f32)
    nc.vector.memset(h.ap(), -5.0)
    nc.const_aps.aps[(f32, -5.0)] = h.ap()
    ones = nc.const_aps.aps[(bf16, 1.0)]  # [128,1] bf16 ones (matmul lhsT)
    nc.all_engine_barrier()

    p_ap = nc.dram_tensor("p", [P, FREE], f32, kind="ExternalInput").ap()
    t_ap = nc.dram_tensor("t", [P, FREE], f32, kind="ExternalInput").ap()
    o_ap = nc.dram_tensor("out", [1, 1024], f32, kind="ExternalOutput").ap()
    oc_ap = None
    if corrections:
        oc_ap = nc.dram_tensor(
            "outc", [P, len(TILE_FS) * len(corrections)], f32,
            kind="ExternalOutput",
        ).ap()

    n_mms = {"o": 0, "u": 0}
    total_mms = sum(f // 512 for f in TILE_FS)

    with contextlib.ExitStack() as es:
        tc = es.enter_context(tile.TileContext(nc))
        io_pool = es.enter_context(tc.tile_pool(name="io", bufs=3))
        tmp = es.enter_context(tc.tile_pool(name="tmp", bufs=2))
        ps_pool = es.enter_context(
            tc.tile_pool(name="ps", bufs=1, space=bass.MemorySpace.PSUM)
        )
        acc_pool = es.enter_context(tc.tile_pool(name="acc", bufs=1))

        psums = {
            "o": ps_pool.tile([1, 512], f32, tag="pso", name="pso"),
            "u": ps_pool.tile([1, 512], f32, tag="psu", name="psu"),
        }
        accs = None
        if corrections:
            accs = acc_pool.tile([P, len(TILE_FS) * len(corrections)], f32)

        def colsum(src_ap, tf, which):
            for c in range(0, tf, 512):
                nc.tensor.matmul(
                    psums[which][:], ones, src_ap[:, c : c + 512],
                    start=(n_mms[which] == 0),
                    stop=(n_mms[which] == total_mms - 1),
                )
                n_mms[which] += 1

        col = 0
        off = 0
        for i, TF in enumerate(TILE_FS):
            sl = slice(off, off + TF)
            off += TF
            pt = io_pool.tile([P, TF], bf16, tag="pt")
            nc.gpsimd.dma_start(out=pt[:], in_=p_ap[:, sl])  # f32->bf16
            tt = io_pool.tile([P, TF], bf16, tag="tt")
            nc.gpsimd.dma_start(out=tt[:], in_=t_ap[:, sl])

            nd = tmp.tile([P, TF], bf16, tag="nd")  # -delta = -5 - 0.05 t
            nc.scalar.activation(nd[:], tt[:], Act.Identity, bias=-5.0, scale=-0.05)
            ws = tmp.tile([P, TF], bf16, tag="ws")  # w_over / 2 = exp(-0.1 t)
            nc.scalar.activation(ws[:], tt[:], Act.Exp, bias=0.0, scale=-0.1)
            wu = tmp.tile([P, TF], bf16, tag="wu")  # w_under = 1 + 0.05 t
            nc.vector.tensor_scalar(
                out=wu[:], in0=tt[:], scalar1=0.05, scalar2=1.0,
                op0=Alu.mult, op1=Alu.add,
            )

            e = tmp.tile([P, TF], bf16, tag="e")
            nc.vector.tensor_tensor(out=e[:], in0=pt[:], in1=tt[:], op=Alu.subtract)
            sh = tmp.tile([P, TF], bf16, tag="sh")  # sign(e) * 2*huber
            nc.vector._custom_dve(HUBER_SIGNED_OP, out=sh[:], in0=e[:], in1=nd[:])
            shp = tmp.tile([P, TF], bf16, tag="shp")  # 2*huber where e>0
            nc.vector.tensor_scalar(
                out=shp[:], in0=sh[:], scalar1=0.0, scalar2=None, op0=Alu.max
            )
            rm = tmp.tile([P, TF], bf16, tag="rm")  # 2*huber where e<0
            nc.vector.tensor_scalar(
                out=rm[:], in0=sh[:], scalar1=-1.0, scalar2=0.0,
                op0=Alu.mult, op1=Alu.max,
            )
            qo = tmp.tile([P, TF], bf16, tag="qo")
            nc.vector.tensor_tensor(out=qo[:], in0=shp[:], in1=ws[:], op=Alu.mult)
            qu = tmp.tile([P, TF], bf16, tag="qu")
            nc.vector.tensor_tensor(out=qu[:], in0=rm[:], in1=wu[:], op=Alu.mult)
            colsum(qo, TF, "o")
            colsum(qu, TF, "u")

            for k, dw in corrections:
                # per-element loss (x2): 2*qo + qu
                qd = tmp.tile([P, TF], bf16, tag="qd")
                nc.vector.scalar_tensor_tensor(
                    out=qd[:], in0=qo[:], scalar=2.0, in1=qu[:],
                    op0=Alu.mult, op1=Alu.add,
                )
                ck = tmp.tile([P, TF], bf16, tag="ck")
                nc.vector.tensor_scalar(
                    out=ck[:], in0=tt[:], scalar1=float(k), scalar2=None,
                    op0=Alu.is_equal,
                )
                qc = tmp.tile([P, TF], bf16, tag="qc")
                nc.vector.scalar_tensor_tensor(
                    out=qc[:], in0=ck[:], scalar=float(dw), in1=qd[:],
                    op0=Alu.mult, op1=Alu.mult,
                    accum_out=accs[:, col : col + 1],
                )
                col += 1
        osb = acc_pool.tile([1, 1024], f32, tag="osb")
        nc.vector.tensor_copy(out=osb[:, 0:512], in_=psums["o"][:])
        nc.vector.tensor_copy(out=osb[:, 512:1024], in_=psums["u"][:])
        nc.sync.dma_start(out=o_ap[:], in_=osb[:])
        if corrections:
            nc.sync.dma_start(out=oc_ap[:], in_=accs[:])
    nc.compile()
    return nc


_cache = {}


def get_nc(corrections):
    key = tuple(corrections)
    if key not in _cache:
        _cache[key] = build(key)
    return _cache[key]


def make_in_maps(predictions, targets):
    p = np.ascontiguousarray(np.asarray(predictions, dtype=np.float32)).reshape(
        NCORES, P, FREE
    )
    t = np.ascontiguousarray(np.asarray(targets, dtype=np.float32)).reshape(
        NCORES, P, FREE
    )
    return [{"p": p[c], "t": t[c]} for c in range(NCORES)]


def freq_corrections(freq_counts):
    fc = np.asarray(freq_counts, dtype=np.float32)
    wf = np.clip(
        np.float32(3.0) / (fc + np.float32(1.0)), np.float32(1.0), np.float32(3.0)
    )
    ks = np.nonzero(wf > 1.0)[0]
    return tuple((int(k), float(wf[k] - 1.0)) for k in ks)


def _run(in_maps, corrections, **kwargs):
    nc = get_nc(corrections)
    return run_bass_kernel_spmd(nc, in_maps, core_ids=list(range(NCORES)), **kwargs)


def reduce_results(res, corrections):
    total = np.float64(0.0)
    for c in range(NCORES):
        o = np.asarray(res.results[c]["out"], dtype=np.float64)
        total += o[0, :512].sum() + 0.5 * o[0, 512:].sum()
        if corrections:
            total += 0.5 * np.asarray(
                res.results[c]["outc"], dtype=np.float64
            ).sum()
    return np.array(total / N, dtype=np.float32)


def kernel(predictions, targets, freq_counts):
    corrections = freq_corrections(freq_counts)
    in_maps = make_in_maps(predictions, targets)
    res = _run(in_maps, corrections)
    return reduce_results(res, corrections)


# revision 27
# speedup vs baseline: 1.2066x; 1.2066x over previous
"""AdaptiveFrequencyAsymmetricHuberLoss on 8 TRN2 NeuronCores (Bass/Tile).

loss = mean( wf(t) * asym(t, sign(e)) * huber(e, delta(t)) ),  e = p - t
  delta(t)   = 5 + 0.05 t
  w_under(t) = 1 + 0.05 t
  w_over(t)  = 2 exp(-t/10)
  wf(t)      = clip(3 / (freq[t] + 1), 1, 3)   (t integer 0..130)
  huber      = 0.5 cl (2e - cl), cl = clip(e, -delta, delta)   (exact identity)

Sharding: pure data parallel; each of the 8 cores streams a contiguous
1/8 of the elements as [128, 16384], DMA-cast f32->bf16 on load.

Per-tile pipeline:
  ACT:  nd = -delta,  ws = w_over (Exp)
  DVE:  e  = p - t                                   [bf16 2x]
        sh = |cl| * (2e - cl) = sign(e) * 2*huber    [8-op custom, 1x]
        shp = max(sh, 0), rm = max(-sh, 0)           [bf16 4x]
        wu = 1 + 0.05 t                              [bf16 4x]
        qo = shp * ws,  qu = rm * wu                 [bf16 2x]
  PE:   ones-colsum matmuls accumulate sum(qo)+sum(qu) into one
        [1,512] PSUM bank across all tiles (only the total matters).
Host divides by 2N and reduces in float64.

The freq table is handled host-side: wf >= 1 always, and wf > 1 only
for freq counts < 2, so the host enumerates the (usually zero) table
entries with wf > 1 and the kernel adds masked correction passes per
entry (accum_out into a separate SBUF accumulator).
"""

import contextlib

import numpy as np

import concourse.bass as bass
import concourse.dve_ops as dve_ops_mod
import concourse.tile as tile
from concourse import bacc, mybir
from concourse.bass_utils import run_bass_kernel_spmd
from concourse.dve_ops import DveOp
from concourse.dve_spec import (
    Spec,
    Src0,
    Src1,
    Zero,
    _has_src1,
    lower,
    maxx,
    minn,
)
from concourse.dve_uop import DveOpSpec

N = 16_777_216
NCORES = 8
P = 128
PER_CORE = N // NCORES          # 2_097_152
FREE = PER_CORE // P            # 16384
TILE_FS = [1024, 3072, 4096, 4096, 3072, 1024]
assert sum(TILE_FS) == FREE

LN2 = 0.6931471805599453

f32 = mybir.dt.float32
bf16 = mybir.dt.bfloat16


def _register_op(name, spec):
    for o in dve_ops_mod.OPS:
        if o.name == name:
            return o
    opcode = max(dve_ops_mod._SUB_OPCODE_FOR_NAME.values()) + 1
    assert opcode < 0x20, "custom-DVE opcode rows exhausted"
    shas = {}
    for ver in ("v3", "v4"):
        try:
            c = DveOpSpec(
                name=name, opcode=opcode, uops=lower(spec, ver=ver),
                rd1_en=_has_src1(spec),
            )
            shas[ver] = c.sha(ver)
        except Exception:
            pass
    op = DveOp(name, spec, subdim=False, uops_sha=shas)
    dve_ops_mod.OPS.append(op)
    dve_ops_mod.CUSTOM_DVE_SPECS[name] = spec
    dve_ops_mod._SUB_OPCODE_FOR_NAME[name] = opcode
    return op


def _huber_signed_ref(in0, in1, c0, c1, c2):
    e = in0.astype(np.float32)
    nd = in1.astype(np.float32)
    cl = np.minimum(np.maximum(e, nd), -nd)
    return (np.abs(cl) * ((e + e) - cl)).astype(np.float32)


# sh = |cl| * (2e - cl) = sign(e) * 2*huber(e, delta);  in0 = e, in1 = -delta
_dd = Zero - Src1
_cl = minn(maxx(Src0, Src1), _dd)
_v = (Src0 + Src0) - _cl
_acl = maxx(_cl, Zero - _cl)
HUBER_SIGNED_SPEC = Spec(
    body=_acl * _v,
    reference=_huber_signed_ref,
)

HUBER_SIGNED_OP = _register_op("HUBER_SIGNED_LOSS_ANT", HUBER_SIGNED_SPEC)


def build(corrections):
    """Build + compile the SPMD graph. corrections: tuple of (k, wf_k - 1)."""
    Alu = mybir.AluOpType
    Act = mybir.ActivationFunctionType

    nc = bacc.Bacc(
        "TRN2", target_bir_lowering=False, debug=False, num_devices=NCORES
    )

    # const AP for the Identity bias (-5); 0.0/1.0 pre-registered by Bass
    h = nc.alloc_sbuf_tensor("const-f32-neg5", [128, 1], f32)
    nc.vector.memset(h.ap(), -5.0)
    nc.const_aps.aps[(f32, -5.0)] = h.ap()
    ones = nc.const_aps.aps[(bf16, 1.0)]  # [128,1] bf16 ones (matmul lhsT)
    nc.all_engine_barrier()

    p_ap = nc.dram_tensor("p", [P, FREE], f32, kind="ExternalInput").ap()
    t_ap = nc.dram_tensor("t", [P, FREE], bf16, kind="ExternalInput").ap()
    o_ap = nc.dram_tensor("out", [1, 1024], f32, kind="ExternalOutput").ap()
    oc_ap = None
    if corrections:
        oc_ap = nc.dram_tensor(
            "outc", [P, len(TILE_FS) * len(corrections)], f32,
            kind="ExternalOutput",
        ).ap()

    n_mms = {"o": 0, "u": 0}
    total_mms = sum(f // 512 for f in TILE_FS)

    with contextlib.ExitStack() as es:
        tc = es.enter_context(tile.TileContext(nc))
        io_pool = es.enter_context(tc.tile_pool(name="io", bufs=3))
        tmp = es.enter_context(tc.tile_pool(name="tmp", bufs=2))
        ps_pool = es.enter_context(
            tc.tile_pool(name="ps", bufs=1, space=bass.MemorySpace.PSUM)
        )
        acc_pool = es.enter_context(tc.tile_pool(name="acc", bufs=1))

        psums = {
            "o": ps_pool.tile([1, 512], f32, tag="pso", name="pso"),
            "u": ps_pool.tile([1, 512], f32, tag="psu", name="psu"),
        }
        accs = None
        if corrections:
            accs = acc_pool.tile([P, len(TILE_FS) * len(corrections)], f32)

        def colsum(src_ap, tf, which):
            for c in range(0, tf, 512):
                nc.tensor.matmul(
                    psums[which][:], ones, src_ap[:, c : c + 512],
                    start=(n_mms[which] == 0),
                    stop=(n_mms[which] == total_mms - 1),
                )
                n_mms[which] += 1

        col = 0
        off = 0
        for i, TF in enumerate(TILE_FS):
            sl = slice(off, off + TF)
            off += TF
            pt = io_pool.tile([P, TF], bf16, tag="pt")
            nc.gpsimd.dma_start(out=pt[:], in_=p_ap[:, sl])  # f32->bf16 cast
            tt = io_pool.tile([P, TF], bf16, tag="tt")
            nc.sync.dma_start(out=tt[:], in_=t_ap[:, sl])    # bf16, HWDGE

            nd = tmp.tile([P, TF], bf16, tag="nd")  # -delta = -5 - 0.05 t
            nc.scalar.activation(nd[:], tt[:], Act.Identity, bias=-5.0, scale=-0.05)
            ws = tmp.tile([P, TF], bf16, tag="ws")  # w_over / 2 = exp(-0.1 t)
            nc.scalar.activation(ws[:], tt[:], Act.Exp, bias=0.0, scale=-0.1)
            wu = tmp.tile([P, TF], bf16, tag="wu")  # w_under = 1 + 0.05 t
            nc.vector.tensor_scalar(
                out=wu[:], in0=tt[:], scalar1=0.05, scalar2=1.0,
                op0=Alu.mult, op1=Alu.add,
            )

            e = tmp.tile([P, TF], bf16, tag="e")
            nc.vector.tensor_tensor(out=e[:], in0=pt[:], in1=tt[:], op=Alu.subtract)
            sh = tmp.tile([P, TF], bf16, tag="sh")  # sign(e) * 2*huber
            nc.vector._custom_dve(HUBER_SIGNED_OP, out=sh[:], in0=e[:], in1=nd[:])
            shp = tmp.tile([P, TF], bf16, tag="shp")  # 2*huber where e>0
            nc.scalar.activation(shp[:], sh[:], Act.Relu, bias=0.0, scale=1.0)
            rm = tmp.tile([P, TF], bf16, tag="rm")  # 2*huber where e<0
            nc.vector.tensor_scalar(
                out=rm[:], in0=sh[:], scalar1=-1.0, scalar2=0.0,
                op0=Alu.mult, op1=Alu.max,
            )
            qo = tmp.tile([P, TF], bf16, tag="qo")
            nc.vector.tensor_tensor(out=qo[:], in0=shp[:], in1=ws[:], op=Alu.mult)
            qu = tmp.tile([P, TF], bf16, tag="qu")
            nc.vector.tensor_tensor(out=qu[:], in0=rm[:], in1=wu[:], op=Alu.mult)
            colsum(qo, TF, "o")
            colsum(qu, TF, "u")

            for k, dw in corrections:
                # per-element loss (x2): 2*qo + qu
                qd = tmp.tile([P, TF], bf16, tag="qd")
                nc.vector.scalar_tensor_tensor(
                    out=qd[:], in0=qo[:], scalar=2.0, in1=qu[:],
                    op0=Alu.mult, op1=Alu.add,
                )
                ck = tmp.tile([P, TF], bf16, tag="ck")
                nc.vector.tensor_scalar(
                    out=ck[:], in0=tt[:], scalar1=float(k), scalar2=None,
                    op0=Alu.is_equal,
                )
                qc = tmp.tile([P, TF], bf16, tag="qc")
                nc.vector.scalar_tensor_tensor(
                    out=qc[:], in0=ck[:], scalar=float(dw), in1=qd[:],
                    op0=Alu.mult, op1=Alu.mult,
                    accum_out=accs[:, col : col + 1],
                )
                col += 1
        osb = acc_pool.tile([1, 1024], f32, tag="osb")
        nc.vector.tensor_copy(out=osb[:, 0:512], in_=psums["o"][:])
        nc.vector.tensor_copy(out=osb[:, 512:1024], in_=psums["u"][:])
        nc.sync.dma_start(out=o_ap[:], in_=osb[:])
        if corrections:
            nc.sync.dma_start(out=oc_ap[:], in_=accs[:])
    nc.compile()
    return nc


_cache = {}


def get_nc(corrections):
    key = tuple(corrections)
    if key not in _cache:
        _cache[key] = build(key)
    return _cache[key]


def make_in_maps(predictions, targets):
    import ml_dtypes

    p = np.ascontiguousarray(np.asarray(predictions, dtype=np.float32)).reshape(
        NCORES, P, FREE
    )
    # targets are integers 0..130: exactly representable in bf16 (lossless)
    t = np.ascontiguousarray(
        np.asarray(targets, dtype=np.float32).astype(ml_dtypes.bfloat16)
    ).reshape(NCORES, P, FREE)
    return [{"p": p[c], "t": t[c]} for c in range(NCORES)]


def freq_corrections(freq_counts):
    fc = np.asarray(freq_counts, dtype=np.float32)
    wf = np.clip(
        np.float32(3.0) / (fc + np.float32(1.0)), np.float32(1.0), np.float32(3.0)
    )
    ks = np.nonzero(wf > 1.0)[0]
    return tuple((int(k), float(wf[k] - 1.0)) for k in ks)


def _run(in_maps, corrections, **kwargs):
    nc = get_nc(corrections)
    return run_bass_kernel_spmd(nc, in_maps, core_ids=list(range(NCORES)), **kwargs)


def reduce_results(res, corrections):
    total = np.float64(0.0)
    for c in range(NCORES):
        o = np.asarray(res.results[c]["out"], dtype=np.float64)
        total += o[0, :512].sum() + 0.5 * o[0, 512:].sum()
        if corrections:
            total += 0.5 * np.asarray(
                res.results[c]["outc"], dtype=np.float64
            ).sum()
    return np.array(total / N, dtype=np.float32)


def kernel(predictions, targets, freq_counts):
    corrections = freq_corrections(freq_counts)
    in_maps = make_in_maps(predictions, targets)
    res = _run(in_maps, corrections)
    return reduce_results(res, corrections)


# revision 30
# speedup vs baseline: 1.2617x; 1.0457x over previous
"""AdaptiveFrequencyAsymmetricHuberLoss on 8 TRN2 NeuronCores (Bass/Tile).

loss = mean( wf(t) * asym(t, sign(e)) * huber(e, delta(t)) ),  e = p - t
  delta(t)   = 5 + 0.05 t
  w_under(t) = 1 + 0.05 t
  w_over(t)  = 2 exp(-t/10)
  wf(t)      = clip(3 / (freq[t] + 1), 1, 3)   (t integer 0..130)
  huber      = 0.5 cl (2e - cl), cl = clip(e, -delta, delta)   (exact identity)

Sharding: pure data parallel; each of the 8 cores streams a contiguous
1/8 of the elements as [128, 16384], DMA-cast f32->bf16 on load.

Per-tile pipeline:
  ACT:  nd = -delta,  ws = w_over (Exp)
  DVE:  e  = p - t                                   [bf16 2x]
        sh = |cl| * (2e - cl) = sign(e) * 2*huber    [8-op custom, 1x]
        shp = max(sh, 0), rm = max(-sh, 0)           [bf16 4x]
        wu = 1 + 0.05 t                              [bf16 4x]
        qo = shp * ws,  qu = rm * wu                 [bf16 2x]
  PE:   ones-colsum matmuls accumulate sum(qo)+sum(qu) into one
        [1,512] PSUM bank across all tiles (only the total matters).
Host divides by 2N and reduces in float64.

The freq table is handled host-side: wf >= 1 always, and wf > 1 only
for freq counts < 2, so the host enumerates the (usually zero) table
entries with wf > 1 and the kernel adds masked correction passes per
entry (accum_out into a separate SBUF accumulator).
"""

import contextlib

import numpy as np

import concourse.bass as bass
import concourse.dve_ops as dve_ops_mod
import concourse.tile as tile
from concourse import bacc, mybir
from concourse.bass_utils import run_bass_kernel_spmd
from concourse.dve_ops import DveOp
from concourse.dve_spec import (
    Spec,
    Src0,
    Src1,
    Zero,
    _has_src1,
    lower,
    maxx,
    minn,
)
from concourse.dve_uop import DveOpSpec

N = 16_777_216
NCORES = 8
P = 128
PER_CORE = N // NCORES          # 2_097_152
FREE = PER_CORE // P            # 16384
TILE_FS = [1024, 3072, 4096, 4096, 3072, 1024]
assert sum(TILE_FS) == FREE

LN2 = 0.6931471805599453

f32 = mybir.dt.float32
bf16 = mybir.dt.bfloat16


def _register_op(name, spec):
    for o in dve_ops_mod.OPS:
        if o.name == name:
            return o
    opcode = max(dve_ops_mod._SUB_OPCODE_FOR_NAME.values()) + 1
    assert opcode < 0x20, "custom-DVE opcode rows exhausted"
    shas = {}
    for ver in ("v3", "v4"):
        try:
            c = DveOpSpec(
                name=name, opcode=opcode, uops=lower(spec, ver=ver),
                rd1_en=_has_src1(spec),
            )
            shas[ver] = c.sha(ver)
        except Exception:
            pass
    op = DveOp(name, spec, subdim=False, uops_sha=shas)
    dve_ops_mod.OPS.append(op)
    dve_ops_mod.CUSTOM_DVE_SPECS[name] = spec
    dve_ops_mod._SUB_OPCODE_FOR_NAME[name] = opcode
    return op


def _huber_signed_ref(in0, in1, c0, c1, c2):
    e = in0.astype(np.float32)
    nd = in1.astype(np.float32)
    cl = np.minimum(np.maximum(e, nd), -nd)
    return (np.abs(cl) * ((e + e) - cl)).astype(np.float32)


# sh = |cl| * (2e - cl) = sign(e) * 2*huber(e, delta);  in0 = e, in1 = -delta
_dd = Zero - Src1
_cl = minn(maxx(Src0, Src1), _dd)
_v = (Src0 + Src0) - _cl
_acl = maxx(_cl, Zero - _cl)
HUBER_SIGNED_SPEC = Spec(
    body=_acl * _v,
    reference=_huber_signed_ref,
)

HUBER_SIGNED_OP = _register_op("HUBER_SIGNED_LOSS_ANT", HUBER_SIGNED_SPEC)


def build(corrections):
    """Build + compile the SPMD graph. corrections: tuple of (k, wf_k - 1)."""
    Alu = mybir.AluOpType
    Act = mybir.ActivationFunctionType

    nc = bacc.Bacc(
        "TRN2", target_bir_lowering=False, debug=False, num_devices=NCORES
    )

    # const AP for the Identity bias (-5); 0.0/1.0 pre-registered by Bass
    h = nc.alloc_sbuf_tensor("const-f32-neg5", [128, 1], f32)
    nc.vector.memset(h.ap(), -5.0)
    nc.const_aps.aps[(f32, -5.0)] = h.ap()
    ones = nc.const_aps.aps[(bf16, 1.0)]  # [128,1] bf16 ones (matmul lhsT)
    nc.all_engine_barrier()

    p_ap = nc.dram_tensor("p", [P, FREE], bf16, kind="ExternalInput").ap()
    t_ap = nc.dram_tensor("t", [P, FREE], bf16, kind="ExternalInput").ap()
    o_ap = nc.dram_tensor("out", [1, 1024], f32, kind="ExternalOutput").ap()
    oc_ap = None
    if corrections:
        oc_ap = nc.dram_tensor(
            "outc", [P, len(TILE_FS) * len(corrections)], f32,
            kind="ExternalOutput",
        ).ap()

    n_mms = {"o": 0, "u": 0}
    total_mms = sum(f // 512 for f in TILE_FS)

    with contextlib.ExitStack() as es:
        tc = es.enter_context(tile.TileContext(nc))
        io_pool = es.enter_context(tc.tile_pool(name="io", bufs=3))
        tmp = es.enter_context(tc.tile_pool(name="tmp", bufs=2))
        ps_pool = es.enter_context(
            tc.tile_pool(name="ps", bufs=1, space=bass.MemorySpace.PSUM)
        )
        acc_pool = es.enter_context(tc.tile_pool(name="acc", bufs=1))

        psums = {
            "o": ps_pool.tile([1, 512], f32, tag="pso", name="pso"),
            "u": ps_pool.tile([1, 512], f32, tag="psu", name="psu"),
        }
        accs = None
        if corrections:
            accs = acc_pool.tile([P, len(TILE_FS) * len(corrections)], f32)

        def colsum(src_ap, tf, which):
            for c in range(0, tf, 512):
                nc.tensor.matmul(
                    psums[which][:], ones, src_ap[:, c : c + 512],
                    start=(n_mms[which] == 0),
                    stop=(n_mms[which] == total_mms - 1),
                )
                n_mms[which] += 1

        col = 0
        off = 0
        for i, TF in enumerate(TILE_FS):
            sl = slice(off, off + TF)
            off += TF
            pt = io_pool.tile([P, TF], bf16, tag="pt")
            nc.sync.dma_start(out=pt[:], in_=p_ap[:, sl])
            tt = io_pool.tile([P, TF], bf16, tag="tt")
            nc.sync.dma_start(out=tt[:], in_=t_ap[:, sl])

            nd = tmp.tile([P, TF], bf16, tag="nd")  # -delta = -5 - 0.05 t
            nc.scalar.activation(nd[:], tt[:], Act.Identity, bias=-5.0, scale=-0.05)
            ws = tmp.tile([P, TF], bf16, tag="ws")  # w_over / 2 = exp(-0.1 t)
            nc.scalar.activation(ws[:], tt[:], Act.Exp, bias=0.0, scale=-0.1)
            wu = tmp.tile([P, TF], bf16, tag="wu")  # w_under = 1 + 0.05 t
            nc.vector.tensor_scalar(
                out=wu[:], in0=tt[:], scalar1=0.05, scalar2=1.0,
                op0=Alu.mult, op1=Alu.add,
            )

            e = tmp.tile([P, TF], bf16, tag="e")
            nc.vector.tensor_tensor(out=e[:], in0=pt[:], in1=tt[:], op=Alu.subtract)
            sh = tmp.tile([P, TF], bf16, tag="sh")  # sign(e) * 2*huber
            nc.vector._custom_dve(HUBER_SIGNED_OP, out=sh[:], in0=e[:], in1=nd[:])
            shp = tmp.tile([P, TF], bf16, tag="shp")  # 2*huber where e>0
            nc.scalar.activation(shp[:], sh[:], Act.Relu, bias=0.0, scale=1.0)
            rm = tmp.tile([P, TF], bf16, tag="rm")  # 2*huber where e<0
            nc.vector.tensor_scalar(
                out=rm[:], in0=sh[:], scalar1=-1.0, scalar2=0.0,
                op0=Alu.mult, op1=Alu.max,
            )
            qo = tmp.tile([P, TF], bf16, tag="qo")
            nc.vector.tensor_tensor(out=qo[:], in0=shp[:], in1=ws[:], op=Alu.mult)
            qu = tmp.tile([P, TF], bf16, tag="qu")
            nc.vector.tensor_tensor(out=qu[:], in0=rm[:], in1=wu[:], op=Alu.mult)
            colsum(qo, TF, "o")
            colsum(qu, TF, "u")

            for k, dw in corrections:
                # per-element loss (x2): 2*qo + qu
                qd = tmp.tile([P, TF], bf16, tag="qd")
                nc.vector.scalar_tensor_tensor(
                    out=qd[:], in0=qo[:], scalar=2.0, in1=qu[:],
                    op0=Alu.mult, op1=Alu.add,
                )
                ck = tmp.tile([P, TF], bf16, tag="ck")
                nc.vector.tensor_scalar(
                    out=ck[:], in0=tt[:], scalar1=float(k), scalar2=None,
                    op0=Alu.is_equal,
                )
                qc = tmp.tile([P, TF], bf16, tag="qc")
                nc.vector.scalar_tensor_tensor(
                    out=qc[:], in0=ck[:], scalar=float(dw), in1=qd[:],
                    op0=Alu.mult, op1=Alu.mult,
                    accum_out=accs[:, col : col + 1],
                )
                col += 1
        osb = acc_pool.tile([1, 1024], f32, tag="osb")
        nc.vector.tensor_copy(out=osb[:, 0:512], in_=psums["o"][:])
        nc.vector.tensor_copy(out=osb[:, 512:1024], in_=psums["u"][:])
        nc.sync.dma_start(out=o_ap[:], in_=osb[:])
        if corrections:
            nc.sync.dma_start(out=oc_ap[:], in_=accs[:])
    nc.compile()
    return nc


_cache = {}


def get_nc(corrections):
    key = tuple(corrections)
    if key not in _cache:
        _cache[key] = build(key)
    return _cache[key]


def make_in_maps(predictions, targets):
    import ml_dtypes

    # The kernel computes in bf16 either way (the previous version
    # DMA-cast f32->bf16 on load with identical round-to-nearest);
    # converting on the host is numerically identical and halves the
    # bytes DMA'd. Targets are integers 0..130: exact in bf16.
    p = np.ascontiguousarray(
        np.asarray(predictions, dtype=np.float32).astype(ml_dtypes.bfloat16)
    ).reshape(NCORES, P, FREE)
    t = np.ascontiguousarray(
        np.asarray(targets, dtype=np.float32).astype(ml_dtypes.bfloat16)
    ).reshape(NCORES, P, FREE)
    return [{"p": p[c], "t": t[c]} for c in range(NCORES)]


def freq_corrections(freq_counts):
    fc = np.asarray(freq_counts, dtype=np.float32)
    wf = np.clip(
        np.float32(3.0) / (fc + np.float32(1.0)), np.float32(1.0), np.float32(3.0)
    )
    ks = np.nonzero(wf > 1.0)[0]
    return tuple((int(k), float(wf[k] - 1.0)) for k in ks)


def _run(in_maps, corrections, **kwargs):
    nc = get_nc(corrections)
    return run_bass_kernel_spmd(nc, in_maps, core_ids=list(range(NCORES)), **kwargs)


def reduce_results(res, corrections):
    total = np.float64(0.0)
    for c in range(NCORES):
        o = np.asarray(res.results[c]["out"], dtype=np.float64)
        total += o[0, :512].sum() + 0.5 * o[0, 512:].sum()
        if corrections:
            total += 0.5 * np.asarray(
                res.results[c]["outc"], dtype=np.float64
            ).sum()
    return np.array(total / N, dtype=np.float32)


def kernel(predictions, targets, freq_counts):
    corrections = freq_corrections(freq_counts)
    in_maps = make_in_maps(predictions, targets)
    res = _run(in_maps, corrections)
    return reduce_results(res, corrections)


# revision 40
# speedup vs baseline: 1.3228x; 1.0485x over previous
"""AdaptiveFrequencyAsymmetricHuberLoss on 8 TRN2 NeuronCores (Bass/Tile).

loss = mean( wf(t) * asym(t, sign(e)) * huber(e, delta(t)) ),  e = p - t
  delta(t)   = 5 + 0.05 t
  w_under(t) = 1 + 0.05 t
  w_over(t)  = 2 exp(-t/10)
  wf(t)      = clip(3 / (freq[t] + 1), 1, 3)   (t integer 0..130)
  huber      = 0.5 cl (2e - cl), cl = clip(e, -delta, delta)   (exact identity)

Sharding: pure data parallel; each of the 8 cores streams a contiguous
1/8 of the elements as [128, 16384], DMA-cast f32->bf16 on load.

Per-tile pipeline:
  ACT:  nd = -delta,  ws = w_over (Exp)
  DVE:  e  = p - t                                   [bf16 2x]
        sh = |cl| * (2e - cl) = sign(e) * 2*huber    [8-op custom, 1x]
        shp = max(sh, 0), rm = max(-sh, 0)           [bf16 4x]
        wu = 1 + 0.05 t                              [bf16 4x]
        qo = shp * ws,  qu = rm * wu                 [bf16 2x]
  PE:   ones-colsum matmuls accumulate sum(qo)+sum(qu) into one
        [1,512] PSUM bank across all tiles (only the total matters).
Host divides by 2N and reduces in float64.

The freq table is handled host-side: wf >= 1 always, and wf > 1 only
for freq counts < 2, so the host enumerates the (usually zero) table
entries with wf > 1 and the kernel adds masked correction passes per
entry (accum_out into a separate SBUF accumulator).
"""

import contextlib

import numpy as np

import concourse.bass as bass
import concourse.dve_ops as dve_ops_mod
import concourse.tile as tile
from concourse import bacc, mybir
from concourse.bass_utils import run_bass_kernel_spmd
from concourse.dve_ops import DveOp
from concourse.dve_spec import (
    Spec,
    Src0,
    Src1,
    Zero,
    _has_src1,
    lower,
    maxx,
    minn,
)
from concourse.dve_uop import DveOpSpec

N = 16_777_216
NCORES = 8
P = 128
PER_CORE = N // NCORES          # 2_097_152
FREE = PER_CORE // P            # 16384
TILE_FS = [1024, 3072, 4096, 4096, 3072, 1024]
assert sum(TILE_FS) == FREE

LN2 = 0.6931471805599453

f32 = mybir.dt.float32
bf16 = mybir.dt.bfloat16


def _register_op(name, spec):
    for o in dve_ops_mod.OPS:
        if o.name == name:
            return o
    opcode = max(dve_ops_mod._SUB_OPCODE_FOR_NAME.values()) + 1
    assert opcode < 0x20, "custom-DVE opcode rows exhausted"
    shas = {}
    for ver in ("v3", "v4"):
        try:
            c = DveOpSpec(
                name=name, opcode=opcode, uops=lower(spec, ver=ver),
                rd1_en=_has_src1(spec),
            )
            shas[ver] = c.sha(ver)
        except Exception:
            pass
    op = DveOp(name, spec, subdim=False, uops_sha=shas)
    dve_ops_mod.OPS.append(op)
    dve_ops_mod.CUSTOM_DVE_SPECS[name] = spec
    dve_ops_mod._SUB_OPCODE_FOR_NAME[name] = opcode
    return op


def _huber_signed_ref(in0, in1, c0, c1, c2):
    e = in0.astype(np.float32)
    nd = in1.astype(np.float32)
    cl = np.minimum(np.maximum(e, nd), -nd)
    return (np.abs(cl) * ((e + e) - cl)).astype(np.float32)


# sh = |cl| * (2e - cl) = sign(e) * 2*huber(e, delta);  in0 = e, in1 = -delta
_dd = Zero - Src1
_cl = minn(maxx(Src0, Src1), _dd)
_v = (Src0 + Src0) - _cl
_acl = maxx(_cl, Zero - _cl)
HUBER_SIGNED_SPEC = Spec(
    body=_acl * _v,
    reference=_huber_signed_ref,
)

HUBER_SIGNED_OP = _register_op("HUBER_SIGNED_LOSS_ANT", HUBER_SIGNED_SPEC)


def build(corrections):
    """Build + compile the SPMD graph. corrections: tuple of (k, wf_k - 1)."""
    Alu = mybir.AluOpType
    Act = mybir.ActivationFunctionType

    nc = bacc.Bacc(
        "TRN2", target_bir_lowering=False, debug=False, num_devices=NCORES
    )

    # const AP for the Identity bias (-5); 0.0/1.0 pre-registered by Bass
    h = nc.alloc_sbuf_tensor("const-f32-neg5", [128, 1], f32)
    nc.vector.memset(h.ap(), -5.0)
    nc.const_aps.aps[(f32, -5.0)] = h.ap()
    ones = nc.const_aps.aps[(bf16, 1.0)]  # [128,1] bf16 ones (matmul lhsT)
    nc.all_engine_barrier()

    p_ap = nc.dram_tensor("p", [P, FREE], bf16, kind="ExternalInput").ap()
    t_ap = nc.dram_tensor("t", [P, FREE], bf16, kind="ExternalInput").ap()
    # chunks of 512: 0 = sum(qo), 1 = sum(rm) [sum(qu) with corrections],
    # 2 = sum(rm*t)
    o_ap = nc.dram_tensor("out", [1, 1536], f32, kind="ExternalOutput").ap()
    oc_ap = None
    if corrections:
        oc_ap = nc.dram_tensor(
            "outc", [P, len(TILE_FS) * len(corrections)], f32,
            kind="ExternalOutput",
        ).ap()

    n_mms = {0: 0, 1: 0, 2: 0}
    total_mms = sum(f // 512 for f in TILE_FS)

    with contextlib.ExitStack() as es:
        tc = es.enter_context(tile.TileContext(nc))
        io_pool = es.enter_context(tc.tile_pool(name="io", bufs=3))
        tmp = es.enter_context(tc.tile_pool(name="tmp", bufs=2))
        ps_pool = es.enter_context(
            tc.tile_pool(name="ps", bufs=1, space=bass.MemorySpace.PSUM)
        )
        acc_pool = es.enter_context(tc.tile_pool(name="acc", bufs=1))

        psrows = [
            ps_pool.tile([1, 512], f32, tag=f"ps{r}", name=f"ps{r}")
            for r in range(3)
        ]
        accs = None
        if corrections:
            accs = acc_pool.tile([P, len(TILE_FS) * len(corrections)], f32)

        def colsum(src_ap, tf, row):
            for c in range(0, tf, 512):
                nc.tensor.matmul(
                    psrows[row][:], ones, src_ap[:, c : c + 512],
                    start=(n_mms[row] == 0),
                    stop=(n_mms[row] == total_mms - 1),
                )
                n_mms[row] += 1

        col = 0
        off = 0
        for i, TF in enumerate(TILE_FS):
            sl = slice(off, off + TF)
            off += TF
            pt = io_pool.tile([P, TF], bf16, tag="pt")
            nc.sync.dma_start(out=pt[:], in_=p_ap[:, sl])
            tt = io_pool.tile([P, TF], bf16, tag="tt")
            nc.sync.dma_start(out=tt[:], in_=t_ap[:, sl])

            nd = tmp.tile([P, TF], bf16, tag="nd")  # -delta = -5 - 0.05 t
            nc.scalar.activation(nd[:], tt[:], Act.Identity, bias=-5.0, scale=-0.05)
            ws = tmp.tile([P, TF], bf16, tag="ws")  # w_over / 2 = exp(-0.1 t)
            nc.scalar.activation(ws[:], tt[:], Act.Exp, bias=0.0, scale=-0.1)
            wu = None
            if corrections:
                wu = tmp.tile([P, TF], bf16, tag="wu")  # w_under = 1 + 0.05 t
                nc.vector.tensor_scalar(
                    out=wu[:], in0=tt[:], scalar1=0.05, scalar2=1.0,
                    op0=Alu.mult, op1=Alu.add,
                )

            e = tmp.tile([P, TF], bf16, tag="e")
            nc.vector.tensor_tensor(out=e[:], in0=pt[:], in1=tt[:], op=Alu.subtract)
            sh = tmp.tile([P, TF], bf16, tag="sh")  # sign(e) * 2*huber
            nc.vector._custom_dve(HUBER_SIGNED_OP, out=sh[:], in0=e[:], in1=nd[:])
            shp = tmp.tile([P, TF], bf16, tag="shp")  # 2*huber where e>0
            nc.scalar.activation(shp[:], sh[:], Act.Relu, bias=0.0, scale=1.0)
            rm = tmp.tile([P, TF], bf16, tag="rm")  # 2*huber where e<0
            nc.vector.tensor_scalar(
                out=rm[:], in0=sh[:], scalar1=-1.0, scalar2=0.0,
                op0=Alu.mult, op1=Alu.max,
            )
            qo = tmp.tile([P, TF], bf16, tag="qo")
            nc.vector.tensor_tensor(out=qo[:], in0=shp[:], in1=ws[:], op=Alu.mult)
            colsum(qo, TF, 0)
            if not corrections:
                # sum(rm*wu) = sum(rm) + 0.05*sum(rm*t): skip the wu tensor
                colsum(rm, TF, 1)
                rmt = tmp.tile([P, TF], bf16, tag="rmt")
                nc.vector.tensor_tensor(out=rmt[:], in0=rm[:], in1=tt[:], op=Alu.mult)
                colsum(rmt, TF, 2)
            else:
                qu = tmp.tile([P, TF], bf16, tag="qu")
                nc.vector.tensor_tensor(out=qu[:], in0=rm[:], in1=wu[:], op=Alu.mult)
                colsum(qu, TF, 1)

            for k, dw in corrections:
                # per-element loss (x2): 2*qo + qu
                qd = tmp.tile([P, TF], bf16, tag="qd")
                nc.vector.scalar_tensor_tensor(
                    out=qd[:], in0=qo[:], scalar=2.0, in1=qu[:],
                    op0=Alu.mult, op1=Alu.add,
                )
                ck = tmp.tile([P, TF], bf16, tag="ck")
                nc.vector.tensor_scalar(
                    out=ck[:], in0=tt[:], scalar1=float(k), scalar2=None,
                    op0=Alu.is_equal,
                )
                qc = tmp.tile([P, TF], bf16, tag="qc")
                nc.vector.scalar_tensor_tensor(
                    out=qc[:], in0=ck[:], scalar=float(dw), in1=qd[:],
                    op0=Alu.mult, op1=Alu.mult,
                    accum_out=accs[:, col : col + 1],
                )
                col += 1
        nrows = 3 if not corrections else 2
        osb = acc_pool.tile([1, 1536], f32, tag="osb")
        for r in range(nrows):
            nc.vector.tensor_copy(
                out=osb[:, r * 512 : (r + 1) * 512], in_=psrows[r][:]
            )
        nc.sync.dma_start(
            out=o_ap[:, : nrows * 512], in_=osb[:, : nrows * 512]
        )
        if corrections:
            nc.sync.dma_start(out=oc_ap[:], in_=accs[:])
    nc.compile()
    return nc


_cache = {}


def get_nc(corrections):
    key = tuple(corrections)
    if key not in _cache:
        _cache[key] = build(key)
    return _cache[key]


def make_in_maps(predictions, targets):
    import ml_dtypes

    # The kernel computes in bf16 either way (the previous version
    # DMA-cast f32->bf16 on load with identical round-to-nearest);
    # converting on the host is numerically identical and halves the
    # bytes DMA'd. Targets are integers 0..130: exact in bf16.
    p = np.ascontiguousarray(
        np.asarray(predictions, dtype=np.float32).astype(ml_dtypes.bfloat16)
    ).reshape(NCORES, P, FREE)
    t = np.ascontiguousarray(
        np.asarray(targets, dtype=np.float32).astype(ml_dtypes.bfloat16)
    ).reshape(NCORES, P, FREE)
    return [{"p": p[c], "t": t[c]} for c in range(NCORES)]


def freq_corrections(freq_counts):
    fc = np.asarray(freq_counts, dtype=np.float32)
    wf = np.clip(
        np.float32(3.0) / (fc + np.float32(1.0)), np.float32(1.0), np.float32(3.0)
    )
    ks = np.nonzero(wf > 1.0)[0]
    return tuple((int(k), float(wf[k] - 1.0)) for k in ks)


def _run(in_maps, corrections, **kwargs):
    nc = get_nc(corrections)
    return run_bass_kernel_spmd(nc, in_maps, core_ids=list(range(NCORES)), **kwargs)


def reduce_results(res, corrections):
    total = np.float64(0.0)
    for c in range(NCORES):
        o = np.asarray(res.results[c]["out"], dtype=np.float64).reshape(3, 512)
        if corrections:
            # chunk0 = sum(qo); chunk1 = sum(rm*wu)
            total += o[0].sum() + 0.5 * o[1].sum()
            total += 0.5 * np.asarray(
                res.results[c]["outc"], dtype=np.float64
            ).sum()
        else:
            # chunk0 = sum(qo); under = 0.5*(sum(rm) + 0.05*sum(rm*t))
            total += o[0].sum() + 0.5 * (o[1].sum() + 0.05 * o[2].sum())
    return np.array(total / N, dtype=np.float32)


def kernel(predictions, targets, freq_counts):
    corrections = freq_corrections(freq_counts)
    in_maps = make_in_maps(predictions, targets)
    res = _run(in_maps, corrections)
    return reduce_results(res, corrections)
